# revision 5
# baseline (speedup 1.0000x reference)
"""Cached multi-head attention decode kernel for 8 trn2 NeuronCores.

Sharding: data-parallel over batch (B=32 -> 4 per core). Each core runs the
full QKV projection, cached attention, and output projection for its 4
batches. No collectives; host concatenates the per-core outputs.

Math (per core, b in 0..3, h in 0..15):
  Q/Kn/Vn = x @ W.T + b        (PE matmuls; weights host-pre-transposed)
  scores[s] = K[s] . q / sqrt(HD)   with K[pos] replaced by Kn
  attn = softmax(scores[0..pos])
  av   = attn @ V                    with V[pos] replaced by Vn
  y    = av_concat @ Wo.T + bo
"""

import sys

if "/opt/trn_rl_repo" not in sys.path:
    sys.path.insert(0, "/opt/trn_rl_repo")

import numpy as np

import concourse.bass as bass  # noqa: F401  (AP helpers)
import concourse.bass_isa as bass_isa
import concourse.mybir as mybir
import concourse.tile as tile
from concourse import bacc
from concourse.bass_utils import run_bass_kernel_spmd
from concourse.masks import make_identity

F32 = mybir.dt.float32

B, S, D, H, HD = 32, 2048, 2048, 16, 128
N_CORES = 8
NB = B // N_CORES          # batches per core
ICH = D // 128             # input-dim chunks of 128
OCN = 512                  # matmul moving-dim tile (psum bank)
SCALE = 1.0 / float(np.sqrt(HD))

_cache = {}


def _install_ntff_shim():
    """antenv.axon_hooks is missing in this image; register the ctypes NTFF
    hook from trn_agent_boot so trace=True works."""
    import types

    try:
        from antenv import axon_hooks  # noqa: F401
        return
    except ImportError:
        pass
    try:
        from trn_agent_boot.trn_boot import _ntff_profile_via_ctypes
        hook = _ntff_profile_via_ctypes("/opt/axon/libaxon_pjrt.so")
    except Exception:
        hook = None
    mod = types.ModuleType("antenv.axon_hooks")
    mod._hook = hook
    mod.get_axon_ntff_profile_hook = lambda: mod._hook

    def _set(h):
        mod._hook = h

    mod.set_axon_ntff_profile_hook = _set
    sys.modules["antenv.axon_hooks"] = mod
    import antenv

    antenv.axon_hooks = mod


def _build(position, nb=NB, nh=H, d=D, s_len=S):
    """Build + compile the per-core program (trace-time specialized on
    `position`)."""
    hd = HD
    ich = d // 128
    ocn_sz = min(OCN, d)
    ocn = d // ocn_sz
    L = position + 1
    nch = (L + 127) // 128
    lc = L - 128 * (nch - 1)          # rows in last chunk (1..128)
    pch, prow = position // 128, position % 128

    nc = bacc.Bacc("TRN2", target_bir_lowering=False, debug=False,
                   num_devices=N_CORES)

    q_d = nc.dram_tensor("q", [nb, d], F32, kind="ExternalInput").ap()
    k_d = nc.dram_tensor("k", [nb, d], F32, kind="ExternalInput").ap()
    v_d = nc.dram_tensor("v", [nb, d], F32, kind="ExternalInput").ap()
    kc_d = nc.dram_tensor("kc", [nb, nh, s_len, hd], F32,
                          kind="ExternalInput").ap()
    vc_d = nc.dram_tensor("vc", [nb, nh, s_len, hd], F32,
                          kind="ExternalInput").ap()
    wqt_d = nc.dram_tensor("wqt", [d, d], F32, kind="ExternalInput").ap()
    wkt_d = nc.dram_tensor("wkt", [d, d], F32, kind="ExternalInput").ap()
    wvt_d = nc.dram_tensor("wvt", [d, d], F32, kind="ExternalInput").ap()
    wot_d = nc.dram_tensor("wot", [d, d], F32, kind="ExternalInput").ap()
    bq_d = nc.dram_tensor("bq", [1, d], F32, kind="ExternalInput").ap()
    bk_d = nc.dram_tensor("bk", [1, d], F32, kind="ExternalInput").ap()
    bv_d = nc.dram_tensor("bv", [1, d], F32, kind="ExternalInput").ap()
    bo_d = nc.dram_tensor("bo", [1, d], F32, kind="ExternalInput").ap()
    y_d = nc.dram_tensor("y", [nb, d], F32, kind="ExternalOutput").ap()

    with tile.TileContext(nc) as tc:
        with (
            tc.tile_pool(name="const", bufs=1) as cpool,
            tc.tile_pool(name="persist", bufs=1) as ppool,
        ):
            ident = cpool.tile([128, 128], F32)
            make_identity(nc, ident[:, :])
            ones = cpool.tile([1, nb], F32)
            nc.vector.memset(ones[:, :], 1.0)

            bias_sb = {}
            for nm_, bd in (("bq", bq_d), ("bk", bk_d), ("bv", bv_d),
                            ("bo", bo_d)):
                t = cpool.tile([1, d], F32, tag=f"bias_{nm_}")
                nc.scalar.dma_start(t[:, :], bd[:, :])
                bias_sb[nm_] = t

            # persistent intermediates
            qt_all = ppool.tile([128, nh * nb], F32, tag="qt")   # q^T cols
            knt_all = ppool.tile([128, nh * nb], F32, tag="knt")  # k_new^T
            vn_nat = ppool.tile([nb, d], F32, tag="vn")           # v_new rows
            avt_all = ppool.tile([128, nh * nb], F32, tag="avt")  # attn-out^T
            y_sb = ppool.tile([nb, d], F32, tag="ysb")

            # ---------------- Phase A: QKV projections ----------------
            with (
                tc.tile_pool(name="a_sbuf", bufs=2) as apool,
                tc.tile_pool(name="a_w", bufs=2) as awpool,
                tc.tile_pool(name="a_tp", bufs=2, space="PSUM") as atpp,
                tc.tile_pool(name="a_pp", bufs=4, space="PSUM") as appp,
            ):
                # load x inputs and transpose to xT chunks [128, nb] each
                xts = {}
                for nm_, xd in (("q", q_d), ("k", k_d), ("v", v_d)):
                    xin = apool.tile([nb, d], F32, tag="xin")
                    nc.sync.dma_start(xin[:, :], xd[:, :])
                    xt = ppool.tile([128, ich * nb], F32, tag=f"xt_{nm_}")
                    for c in range(ich):
                        pt = atpp.tile([128, nb], F32, tag="tp")
                        nc.tensor.transpose(
                            pt[:, :], xin[:, c * 128:(c + 1) * 128],
                            ident[0:nb, 0:nb])
                        nc.vector.tensor_copy(
                            xt[:, c * nb:(c + 1) * nb], pt[:, :])
                    xts[nm_] = xt

                # projections: out_nat [nb, d] = x @ W.T + b
                for nm_, wd, bnm in (("q", wqt_d, "bq"), ("k", wkt_d, "bk"),
                                     ("v", wvt_d, "bv")):
                    xt = xts[nm_]
                    psums = [appp.tile([nb, ocn_sz], F32, tag="pp", name=f"pp{_oc}")
                             for _oc in range(ocn)]
                    for c in range(ich):
                        wc = awpool.tile([128, d], F32, tag="wch")
                        nc.scalar.dma_start(
                            wc[:, :], wd[c * 128:(c + 1) * 128, :])
                        for oc in range(ocn):
                            nc.tensor.matmul(
                                psums[oc][:, :],
                                lhsT=xt[:, c * nb:(c + 1) * nb],
                                rhs=wc[:, oc * ocn_sz:(oc + 1) * ocn_sz],
                                start=(c == 0), stop=False)
                    for oc in range(ocn):
                        nc.tensor.matmul(
                            psums[oc][:, :], lhsT=ones[:, :],
                            rhs=bias_sb[bnm][:, oc * ocn_sz:(oc + 1) * ocn_sz],
                            start=False, stop=True)

                    if nm_ == "v":
                        for oc in range(ocn):
                            nc.vector.tensor_copy(
                                vn_nat[:, oc * ocn_sz:(oc + 1) * ocn_sz],
                                psums[oc][:, :])
                    else:
                        xnat = apool.tile([nb, d], F32, tag="xnat")
                        for oc in range(ocn):
                            nc.vector.tensor_copy(
                                xnat[:, oc * ocn_sz:(oc + 1) * ocn_sz],
                                psums[oc][:, :])
                        dst = qt_all if nm_ == "q" else knt_all
                        for h in range(nh):
                            pt = atpp.tile([128, nb], F32, tag="tp")
                            nc.tensor.transpose(
                                pt[:, :], xnat[:, h * 128:(h + 1) * 128],
                                ident[0:nb, 0:nb])
                            nc.vector.tensor_copy(
                                dst[:, h * nb:(h + 1) * nb], pt[:, :])

            # ---------------- Phase B: attention ----------------
            with (
                tc.tile_pool(name="b_k", bufs=2) as kpool,
                tc.tile_pool(name="b_v", bufs=2) as vpool,
                tc.tile_pool(name="b_kt", bufs=2) as ktpool,
                tc.tile_pool(name="b_sm", bufs=3) as smpool,
                tc.tile_pool(name="b_es", bufs=2) as espool,
                tc.tile_pool(name="b_ktp", bufs=3, space="PSUM") as ktpp,
                tc.tile_pool(name="b_sp", bufs=2, space="PSUM") as spp,
                tc.tile_pool(name="b_av", bufs=2, space="PSUM") as avpp,
            ):
                for h in range(nh):
                    for b in range(nb):
                        col = h * nb + b
                        # stream K and V slabs: [128 part = s%128, chunk, hd]
                        ktile = kpool.tile([128, nch, hd], F32, tag="ktile")
                        nc.sync.dma_start(
                            ktile[:, :, :],
                            kc_d[b, h, 0:nch * 128, :].rearrange(
                                "(c p) e -> p c e", p=128))
                        vtile = vpool.tile([128, nch, hd], F32, tag="vtile")
                        nc.sync.dma_start(
                            vtile[:, :, :],
                            vc_d[b, h, 0:nch * 128, :].rearrange(
                                "(c p) e -> p c e", p=128))
                        # splice v_new into V at `position` (SWDGE ring so the
                        # wait-for-phase-A doesn't stall the sync HWDGE ring)
                        nc.gpsimd.dma_start(
                            vtile[prow:prow + 1, pch, :],
                            vn_nat[b:b + 1, h * 128:(h + 1) * 128])

                        # K^T via PE transposes; k_new spliced at `position`
                        kt = ktpool.tile([128, nch * 128], F32, tag="kt")
                        for c in range(nch):
                            pt = ktpp.tile([128, 128], F32, tag="ktp")
                            nc.tensor.transpose(
                                pt[:, :], ktile[:, c, :], ident[:, :])
                            if c == pch:
                                if prow > 0:
                                    nc.vector.tensor_copy(
                                        kt[:, c * 128:c * 128 + prow],
                                        pt[:, 0:prow])
                                if prow < 127:
                                    nc.vector.tensor_copy(
                                        kt[:, c * 128 + prow + 1:
                                           (c + 1) * 128],
                                        pt[:, prow + 1:128])
                            else:
                                nc.vector.tensor_copy(
                                    kt[:, c * 128:(c + 1) * 128], pt[:, :])
                        nc.vector.tensor_copy(
                            kt[:, position:position + 1],
                            knt_all[:, col:col + 1])

                        # scores[s%128, s//128] = K[s] . q
                        sp = spp.tile([128, nch], F32, tag="sp")
                        if lc < 128:
                            nc.vector.memset(sp[:, :], -1e30)
                        for c in range(nch):
                            m = 128 if c < nch - 1 else lc
                            nc.tensor.matmul(
                                sp[0:m, c:c + 1],
                                lhsT=kt[:, c * 128:c * 128 + m],
                                rhs=qt_all[:, col:col + 1],
                                start=True, stop=True)
                        # softmax (denominator folded into AV evacuation)
                        m1 = smpool.tile([128, 1], F32, tag="m1")
                        nc.vector.reduce_max(m1[:, :], sp[:, :],
                                             axis=mybir.AxisListType.X)
                        mall = smpool.tile([128, 1], F32, tag="mall")
                        nc.gpsimd.partition_all_reduce(
                            mall[:, :], m1[:, :], channels=128,
                            reduce_op=bass_isa.ReduceOp.max)
                        nmx = smpool.tile([128, 1], F32, tag="nmx")
                        nc.vector.tensor_scalar_mul(nmx[:, :], mall[:, :],
                                                    -SCALE)
                        es = espool.tile([128, nch], F32, tag="es")
                        s1 = smpool.tile([128, 1], F32, tag="s1")
                        nc.scalar.activation(
                            es[:, :], sp[:, :],
                            mybir.ActivationFunctionType.Exp,
                            bias=nmx[:, 0:1], scale=SCALE,
                            accum_out=s1[:, 0:1])
                        sall = smpool.tile([128, 1], F32, tag="sall")
                        nc.gpsimd.partition_all_reduce(
                            sall[:, :], s1[:, :], channels=128,
                            reduce_op=bass_isa.ReduceOp.add)
                        rcp = smpool.tile([128, 1], F32, tag="rcp")
                        nc.vector.reciprocal(rcp[:, :], sall[:, :])

                        # av = sum_s attn[s] * V[s]
                        avp = avpp.tile([128, 1], F32, tag="avp")
                        for c in range(nch):
                            m = 128 if c < nch - 1 else lc
                            nc.tensor.matmul(
                                avp[:, :], lhsT=vtile[0:m, c, :],
                                rhs=es[0:m, c:c + 1],
                                start=(c == 0), stop=(c == nch - 1))
                        nc.vector.tensor_scalar_mul(
                            avt_all[:, col:col + 1], avp[:, :], rcp[:, 0:1])

            # ---------------- Phase C: output projection ----------------
            with (
                tc.tile_pool(name="c_w", bufs=2) as cwpool,
                tc.tile_pool(name="c_pp", bufs=4, space="PSUM") as cppp,
            ):
                psums = [cppp.tile([nb, ocn_sz], F32, tag="cpp", name=f"cpp{_oc}")
                         for _oc in range(ocn)]
                for h in range(nh):
                    wc = cwpool.tile([128, d], F32, tag="wo")
                    nc.scalar.dma_start(wc[:, :],
                                        wot_d[h * 128:(h + 1) * 128, :])
                    for oc in range(ocn):
                        nc.tensor.matmul(
                            psums[oc][:, :],
                            lhsT=avt_all[:, h * nb:(h + 1) * nb],
                            rhs=wc[:, oc * ocn_sz:(oc + 1) * ocn_sz],
                            start=(h == 0), stop=False)
                for oc in range(ocn):
                    nc.tensor.matmul(
                        psums[oc][:, :], lhsT=ones[:, :],
                        rhs=bias_sb["bo"][:, oc * ocn_sz:(oc + 1) * ocn_sz],
                        start=False, stop=True)
                for oc in range(ocn):
                    nc.vector.tensor_copy(
                        y_sb[:, oc * ocn_sz:(oc + 1) * ocn_sz],
                        psums[oc][:, :])
                nc.sync.dma_start(y_d[:, :], y_sb[:, :])

    nc.compile()
    return nc


def _get_nc(position):
    if position not in _cache:
        _cache[position] = _build(position)
    return _cache[position]


def _make_in_maps(inputs):
    f = lambda a: np.ascontiguousarray(np.asarray(a), dtype=np.float32)
    wqt = f(np.asarray(inputs["Wq"]).T)
    wkt = f(np.asarray(inputs["Wk"]).T)
    wvt = f(np.asarray(inputs["Wv"]).T)
    wot = f(np.asarray(inputs["Wo"]).T)
    bq = f(inputs["bq"]).reshape(1, D)
    bk = f(inputs["bk"]).reshape(1, D)
    bv = f(inputs["bv"]).reshape(1, D)
    bo = f(inputs["bo"]).reshape(1, D)
    q = f(inputs["query"]).reshape(B, D)
    k = f(inputs["key"]).reshape(B, D)
    v = f(inputs["value"]).reshape(B, D)
    kc = np.asarray(inputs["key_cache"])
    vc = np.asarray(inputs["value_cache"])
    in_maps = []
    for i in range(N_CORES):
        sl = slice(i * NB, (i + 1) * NB)
        in_maps.append({
            "q": q[sl], "k": k[sl], "v": v[sl],
            "kc": f(kc[sl]), "vc": f(vc[sl]),
            "wqt": wqt, "wkt": wkt, "wvt": wvt, "wot": wot,
            "bq": bq, "bk": bk, "bv": bv, "bo": bo,
        })
    return in_maps


def _run(inputs, trace=False):
    position = int(inputs["position"])
    if trace:
        _install_ntff_shim()
    nc = _get_nc(position)
    in_maps = _make_in_maps(inputs)
    res = run_bass_kernel_spmd(nc, in_maps, list(range(N_CORES)), trace=trace)
    out = np.concatenate([res.results[i]["y"] for i in range(N_CORES)],
                         axis=0).reshape(B, 1, D)
    return out, res


def kernel(**inputs):
    out, _ = _run(inputs, trace=False)
    return out
